# revision 3
# baseline (speedup 1.0000x reference)
"""MaxUnpooling2D scatter kernel for Trainium2 (8 NeuronCores, batch-parallel).

Problem: inputs [16,128,128,64] f32, argmax [16,128,128,64] i32 with
per-batch flattened indices into the [256,256,64] output space, laid out as
    argmax = ((2h+dh)*Wo + (2w+dw))*C + c,   dh,dw in {0,1}
Output [16,256,256,64] f32: each input value lands in one cell of its own
2x2 output window; all other cells are 0 (no duplicate indices possible).

Key observation: the bit fields of argmax are disjoint —
    c  = bits 0-5, dw = bit 6, w = bits 7-13, dh = bit 14, h = bits 15-21
so code = argmax & 0x4040 isolates (dh, dw), and each of the four output
slots is a single fused DVE op:
    out[dh][:, :, dw, :] = (code == (dh<<14 | dw<<6)) * v
computed as scalar_tensor_tensor(is_equal, mult).

Sharding: batch dim 16 -> 2 batches per core, fully local, no collectives.
"""

import numpy as np

import concourse.bass as bass
import concourse.mybir as mybir
from concourse.tile import TileContext
from concourse.bass_utils import run_bass_kernel_spmd

# ---- problem constants (hardcoded; kernel.py must be self-contained) ----
B, H, W, C = 16, 128, 128, 64
N_CORES = 8
B_SHARD = B // N_CORES  # 2
Ho, Wo = 2 * H, 2 * W
WC = W * C  # 8192  (free elems per input row)
WoC = Wo * C  # 16384 (free elems per output row)

CHUNK_W = 32  # input columns per chunk
NCH = W // CHUNK_W  # 4 chunks
CHF = CHUNK_W * C  # 2048 free elems per input chunk
CHF2 = 2 * CHF  # 4096 free elems per output-row chunk

_MASK = 0x4040  # dh bit 14 | dw bit 6

# walrus in this container supports only ONE sem-wait per instruction;
# split excess waits onto preceding NoOps at serialization time.
_MAX_WAITS = 1


def _split_waits(bir_json_bytes: bytes) -> bytes:
    import json

    m = json.loads(bir_json_bytes)
    for f in m.get("functions", []):
        for bb in f.get("blocks", []):
            new_instructions = []
            for ins in bb.get("instructions", []):
                sync = ins.get("sync_info")
                waits = (sync or {}).get("on_wait") or []
                if len(waits) > _MAX_WAITS:
                    extra = waits[:-_MAX_WAITS]
                    sync["on_wait"] = waits[-_MAX_WAITS:]
                    for ci, start in enumerate(range(0, len(extra), _MAX_WAITS)):
                        chunk = extra[start : start + _MAX_WAITS]
                        nop = {
                            "engine": ins["engine"],
                            "ins": [],
                            "name": f"{ins['name']}_ws{ci}",
                            "opcode": "NoOp",
                            "outs": [],
                            "sync_info": {"on_update": [], "on_wait": chunk},
                        }
                        if ins.get("debug") is not None:
                            nop["debug"] = ins["debug"]
                        new_instructions.append(nop)
                new_instructions.append(ins)
            bb["instructions"] = new_instructions
    return json.dumps(m).encode()


def _build():
    nc = bass.Bass()
    x = nc.dram_tensor("x", [B_SHARD, H, WC], mybir.dt.float32, kind="ExternalInput")
    idx = nc.dram_tensor("idx", [B_SHARD, H, WC], mybir.dt.int32, kind="ExternalInput")
    out = nc.dram_tensor(
        "out", [B_SHARD, Ho, WoC], mybir.dt.float32, kind="ExternalOutput"
    )

    with TileContext(nc) as tc:
        with tc.tile_pool(name="io", bufs=4) as io_pool, tc.tile_pool(
            name="codep", bufs=2
        ) as code_pool, tc.tile_pool(name="rows", bufs=2) as row_pool:
            for b in range(B_SHARD):
                # out rows r = 2h + dh: [2, 128, WoC] view, partition dim = h
                out_v = out[b].rearrange("(h t) f -> t h f", t=2)
                for j in range(NCH):
                    xt = io_pool.tile([H, CHF], mybir.dt.float32, tag="xt")
                    it = io_pool.tile([H, CHF], mybir.dt.int32, tag="it")
                    nc.sync.dma_start(out=xt[:], in_=x[b][:, j * CHF : (j + 1) * CHF])
                    nc.sync.dma_start(out=it[:], in_=idx[b][:, j * CHF : (j + 1) * CHF])

                    code = code_pool.tile([H, CHF], mybir.dt.int32, tag="code")
                    nc.vector.tensor_scalar(
                        out=code[:],
                        in0=it[:],
                        scalar1=_MASK,
                        scalar2=None,
                        op0=mybir.AluOpType.bitwise_and,
                    )

                    code_v = code[:].rearrange("p (w c) -> p w c", c=C)
                    x_v = xt[:].rearrange("p (w c) -> p w c", c=C)
                    rows = [
                        row_pool.tile(
                            [H, CHF2],
                            mybir.dt.float32,
                            tag=f"row{dh}",
                            name=f"row{dh}_{b}_{j}",
                        )
                        for dh in range(2)
                    ]
                    for dh in range(2):
                        row_v = rows[dh][:].rearrange("p (w t c) -> p w t c", t=2, c=C)
                        for dw in range(2):
                            nc.vector.scalar_tensor_tensor(
                                out=row_v[:, :, dw, :],
                                in0=code_v,
                                scalar=float((dh << 14) | (dw << 6)),
                                in1=x_v,
                                op0=mybir.AluOpType.is_equal,
                                op1=mybir.AluOpType.mult,
                            )
                        nc.scalar.dma_start(
                            out=out_v[dh][:, j * CHF2 : (j + 1) * CHF2],
                            in_=rows[dh][:],
                        )

    # install the wait-split serialization fix
    orig = nc.to_json_bytes

    def patched(*a, **k):
        return _split_waits(orig(*a, **k))

    nc.to_json_bytes = patched
    return nc


_nc_cache = None


def _run(inputs: np.ndarray, argmax: np.ndarray, **spmd_kwargs):
    global _nc_cache
    if _nc_cache is None:
        _nc_cache = _build()
    nc = _nc_cache

    x = np.ascontiguousarray(np.asarray(inputs, dtype=np.float32).reshape(B, H, WC))
    idx = np.ascontiguousarray(np.asarray(argmax, dtype=np.int32).reshape(B, H, WC))

    in_maps = [
        {
            "x": x[i * B_SHARD : (i + 1) * B_SHARD],
            "idx": idx[i * B_SHARD : (i + 1) * B_SHARD],
        }
        for i in range(N_CORES)
    ]
    res = run_bass_kernel_spmd(nc, in_maps, core_ids=list(range(N_CORES)), **spmd_kwargs)
    out = np.concatenate([r["out"] for r in res.results], axis=0)
    return out.reshape(B, Ho, Wo, C), res


def kernel(inputs: np.ndarray, argmax: np.ndarray) -> np.ndarray:
    out, _ = _run(inputs, argmax)
    return out


# revision 8
# speedup vs baseline: 714.9006x; 714.9006x over previous
"""MaxUnpooling2D scatter kernel for Trainium2 (8 NeuronCores, batch-parallel).

Problem: inputs [16,128,128,64] f32, argmax [16,128,128,64] i32 with
per-batch flattened indices into the [256,256,64] output space, laid out as
    argmax = ((2h+dh)*Wo + (2w+dw))*C + c,   dh,dw in {0,1}
Output [16,256,256,64] f32: each input value lands in one cell of its own
2x2 output window; all other cells are 0 (no duplicate indices possible).

Key observation: the bit fields of argmax are disjoint —
    c  = bits 0-5, dw = bit 6, w = bits 7-13, dh = bit 14, h = bits 15-21
so code = argmax & 0x4040 isolates (dh, dw), and each of the four output
slots is a single fused DVE op:
    out[dh][:, :, dw, :] = (code == (dh<<14 | dw<<6)) * v
computed as scalar_tensor_tensor(is_equal, mult).

Sharding: batch dim 16 -> 2 batches per core, fully local, no collectives.
"""

import numpy as np

import concourse.bass as bass
import concourse.mybir as mybir
from concourse.tile import TileContext
from concourse.bass_utils import run_bass_kernel_spmd

# ---- problem constants (hardcoded; kernel.py must be self-contained) ----
B, H, W, C = 16, 128, 128, 64
N_CORES = 8
B_SHARD = B // N_CORES  # 2
Ho, Wo = 2 * H, 2 * W
WC = W * C  # 8192  (free elems per input row)
WoC = Wo * C  # 16384 (free elems per output row)

CHUNK_W = 32  # input columns per chunk
NCH = W // CHUNK_W  # 4 chunks
CHF = CHUNK_W * C  # 2048 free elems per input chunk
CHF2 = 2 * CHF  # 4096 free elems per output-row chunk

_MASK = 0x4040  # dh bit 14 | dw bit 6

# walrus in this container supports only ONE sem-wait per instruction;
# split excess waits onto preceding NoOps at serialization time.
_MAX_WAITS = 1


def _split_waits(bir_json_bytes: bytes) -> bytes:
    import json

    m = json.loads(bir_json_bytes)
    for f in m.get("functions", []):
        for bb in f.get("blocks", []):
            new_instructions = []
            for ins in bb.get("instructions", []):
                sync = ins.get("sync_info")
                waits = (sync or {}).get("on_wait") or []
                if len(waits) > _MAX_WAITS:
                    extra = waits[:-_MAX_WAITS]
                    sync["on_wait"] = waits[-_MAX_WAITS:]
                    for ci, start in enumerate(range(0, len(extra), _MAX_WAITS)):
                        chunk = extra[start : start + _MAX_WAITS]
                        nop = {
                            "engine": ins["engine"],
                            "ins": [],
                            "name": f"{ins['name']}_ws{ci}",
                            "opcode": "NoOp",
                            "outs": [],
                            "sync_info": {"on_update": [], "on_wait": chunk},
                        }
                        if ins.get("debug") is not None:
                            nop["debug"] = ins["debug"]
                        new_instructions.append(nop)
                new_instructions.append(ins)
            bb["instructions"] = new_instructions
    return json.dumps(m).encode()


def _build(pair_store: bool = False):
    # pair_store=True (single fused row-pair store DMA) hit
    # NRT_EXEC_UNIT_UNRECOVERABLE on hardware; keep the two-store variant.
    nc = bass.Bass()
    x = nc.dram_tensor("x", [B_SHARD, H, WC], mybir.dt.float32, kind="ExternalInput")
    idx = nc.dram_tensor("idx", [B_SHARD, H, WC], mybir.dt.int32, kind="ExternalInput")
    out = nc.dram_tensor(
        "out", [B_SHARD, Ho, WoC], mybir.dt.float32, kind="ExternalOutput"
    )

    with TileContext(nc) as tc:
        with tc.tile_pool(name="io", bufs=4) as io_pool, tc.tile_pool(
            name="codep", bufs=2
        ) as code_pool, tc.tile_pool(name="rows", bufs=2) as row_pool:
            for b in range(B_SHARD):
                # out rows r = 2h + dh: [128(h), 2(dh), WoC] view, partition = h
                out_v = out[b].rearrange("(h t) f -> h t f", t=2)
                for j in range(NCH):
                    xt = io_pool.tile([H, CHF], mybir.dt.float32, tag="xt")
                    it = io_pool.tile([H, CHF], mybir.dt.int32, tag="it")
                    nc.sync.dma_start(out=xt[:], in_=x[b][:, j * CHF : (j + 1) * CHF])
                    nc.sync.dma_start(out=it[:], in_=idx[b][:, j * CHF : (j + 1) * CHF])

                    code = code_pool.tile([H, CHF], mybir.dt.int32, tag="code")
                    nc.vector.tensor_scalar(
                        out=code[:],
                        in0=it[:],
                        scalar1=_MASK,
                        scalar2=None,
                        op0=mybir.AluOpType.bitwise_and,
                    )

                    code_v = code[:].rearrange("p (w c) -> p w c", c=C)
                    x_v = xt[:].rearrange("p (w c) -> p w c", c=C)
                    if pair_store:
                        # one tile holding both output rows' chunk:
                        # [128(h), 2(dh), CHUNK_W(w), 2(dw), C]
                        pair = row_pool.tile(
                            [H, 2 * CHF2],
                            mybir.dt.float32,
                            tag="pair",
                            name=f"pair_{b}_{j}",
                        )
                        pair_v = pair[:].rearrange(
                            "p (t w u c) -> p t w u c", t=2, w=CHUNK_W, u=2
                        )
                        for dh in range(2):
                            for dw in range(2):
                                nc.vector.scalar_tensor_tensor(
                                    out=pair_v[:, dh, :, dw, :],
                                    in0=code_v,
                                    scalar=float((dh << 14) | (dw << 6)),
                                    in1=x_v,
                                    op0=mybir.AluOpType.is_equal,
                                    op1=mybir.AluOpType.mult,
                                )
                        # dest: rows 2h,2h+1 cols [j*CHF2, (j+1)*CHF2)
                        nc.scalar.dma_start(
                            out=out_v[:, :, j * CHF2 : (j + 1) * CHF2],
                            in_=pair[:].rearrange("p (t f) -> p t f", t=2),
                        )
                    else:
                        rows = [
                            row_pool.tile(
                                [H, CHF2],
                                mybir.dt.float32,
                                tag=f"row{dh}",
                                name=f"row{dh}_{b}_{j}",
                            )
                            for dh in range(2)
                        ]
                        for dh in range(2):
                            row_v = rows[dh][:].rearrange(
                                "p (w t c) -> p w t c", t=2, c=C
                            )
                            for dw in range(2):
                                nc.vector.scalar_tensor_tensor(
                                    out=row_v[:, :, dw, :],
                                    in0=code_v,
                                    scalar=float((dh << 14) | (dw << 6)),
                                    in1=x_v,
                                    op0=mybir.AluOpType.is_equal,
                                    op1=mybir.AluOpType.mult,
                                )
                            nc.scalar.dma_start(
                                out=out_v[:, dh, j * CHF2 : (j + 1) * CHF2],
                                in_=rows[dh][:],
                            )

    # install the wait-split serialization fix
    orig = nc.to_json_bytes

    def patched(*a, **k):
        return _split_waits(orig(*a, **k))

    nc.to_json_bytes = patched
    return nc


_nc_cache = None


def _run(inputs: np.ndarray, argmax: np.ndarray, **spmd_kwargs):
    global _nc_cache
    if _nc_cache is None:
        _nc_cache = _build()
    nc = _nc_cache

    x = np.ascontiguousarray(np.asarray(inputs, dtype=np.float32).reshape(B, H, WC))
    idx = np.ascontiguousarray(np.asarray(argmax, dtype=np.int32).reshape(B, H, WC))

    in_maps = [
        {
            "x": x[i * B_SHARD : (i + 1) * B_SHARD],
            "idx": idx[i * B_SHARD : (i + 1) * B_SHARD],
        }
        for i in range(N_CORES)
    ]
    res = run_bass_kernel_spmd(nc, in_maps, core_ids=list(range(N_CORES)), **spmd_kwargs)
    out = np.concatenate([r["out"] for r in res.results], axis=0)
    return out.reshape(B, Ho, Wo, C), res


def kernel(inputs: np.ndarray, argmax: np.ndarray) -> np.ndarray:
    out, _ = _run(inputs, argmax)
    return out
